# revision 33
# baseline (speedup 1.0000x reference)
"""HGT layer kernel for 8 Trainium2 NeuronCores (Bass/Tile) — v2.

Design (vs. v1 baseline):
- dst-range sharding: core c owns dst nodes [c*6250, (c+1)*6250); edges bucketed
  per dst-owner, chunk-aligned packing (CAP=192 per 128-node chunk, NSUB=74
  subtiles per relation) so the SPMD program is static across cores.
- Compact kv table: only the ~20k unique src nodes each core actually
  references (host computes uniq + remap); k-half has NO bias (cancels in
  segment softmax), v-half keeps bv (so empty segments stay exactly 0).
- Host pre-transposes x (xgT/xqT) so phase A needs no DMA transposes.
- qa side: NO DRAM table and NO gathers — per-pair qA windows are built
  on-the-fly (PE matmul) and expanded per-edge with one-hot S^T matmuls.
- Node-major segment sums: num [node,256] + 4-wide den via the same one-hot S
  lhsT; division with free-dim broadcast; z transposed back via PE transposes
  for the folded output transform (rel_msg x Wa x alpha/R folded into wt).
- exp via Act engine broadcast to 64-wide so Y multiply runs in DVE 2x mode.
- alpha, ba folded host-side into xb_pre = (1-alpha)x + alpha*ba.
"""
import sys, types
import numpy as np
import ml_dtypes

if "antenv.axon_hooks" not in sys.modules:
    try:
        from trn_agent_boot.trn_boot import _ntff_profile_via_ctypes as _mk_hook
        _m = types.ModuleType("antenv.axon_hooks")
        _m.get_axon_ntff_profile_hook = lambda: None
        sys.modules["antenv.axon_hooks"] = _m
    except Exception:
        pass

import concourse.bass as bass
import concourse.bacc as bacc
import concourse.tile as tile
import concourse.mybir as mybir
from concourse.bass_utils import run_bass_kernel_spmd

BF16 = mybir.dt.bfloat16
F32 = mybir.dt.float32
I32 = mybir.dt.int32
BF = ml_dtypes.bfloat16
F8 = mybir.dt.float8e4
F8N = ml_dtypes.float8_e4m3
Alu = mybir.AluOpType
Act = mybir.ActivationFunctionType

N, D, R, H, DK = 50000, 256, 4, 4, 64
NC_ = 8
NLOC = N // NC_          # 6250
CH = 128
NCHUNK = 49
NPAIR = 25
NSUB = 75                # 74 used + 1 pad so every pair has 3
CAP = 192
NQ = 6400                # padded own-node count
SQRT_DK = 8.0

_cache: dict = {}


def pb4(t):
    """[128, 3, 256] -> [128, 3, 4] view picking element 0 of each 64-block."""
    return t.rearrange("p s (h d) -> p s h d", h=4)[:, :, :, 0]



# ---------------------------------------------------------------- host prep
def _pack_edges(src, dst, core, uniq):
    """Chunk-aligned packing (as v1): returns okv [128, NSUB] int32 (compact
    kv rows), S [128, NSUB, 128] bf16, ST [128, NSUB, 128] bf16."""
    sel = (dst >= core * NLOC) & (dst < (core + 1) * NLOC)
    es = src[sel].astype(np.int64)
    ed = (dst[sel] - core * NLOC).astype(np.int64)
    chunk = ed >> 7
    order = np.lexsort((es, chunk))
    es, ed, chunk = es[order], ed[order], chunk[order]
    counts = np.bincount(chunk, minlength=NCHUNK)
    if counts.max() > CAP:
        raise RuntimeError(f"chunk overflow: {counts.max()} > {CAP}")
    starts = np.zeros(NCHUNK, np.int64)
    starts[1:] = np.cumsum(counts)[:-1]
    slot = np.arange(len(ed)) - starts[chunk]
    P = chunk >> 1
    even = (chunk & 1) == 0
    sub = np.where(even,
                   np.where(slot < 128, 3 * P, 3 * P + 1),
                   np.where(slot < 64, 3 * P + 1, 3 * P + 2))
    part = np.where(even,
                    np.where(slot < 128, slot, slot - 128),
                    np.where(slot < 64, 64 + slot, slot - 64))
    okv = np.zeros((128, NSUB), np.int32)
    S = np.zeros((128, NSUB, 128), np.float32)
    kvrow = np.searchsorted(uniq, es).astype(np.int32)
    okv[part, sub] = kvrow
    S[part, sub, ed & 127] = 1.0
    ST = S.transpose(2, 1, 0).copy()    # [node, sub, edge]
    return okv, S, ST


def _host_prep(inputs):
    x = np.asarray(inputs["x"], np.float32)
    Wk = np.asarray(inputs["Wk"], np.float32)
    Wq, bq = np.asarray(inputs["Wq"], np.float32), np.asarray(inputs["bq"], np.float32)
    Wv, bv = np.asarray(inputs["Wv"], np.float32), np.asarray(inputs["bv"], np.float32)
    Wa, ba = np.asarray(inputs["Wa"], np.float32), np.asarray(inputs["ba"], np.float32)
    rel_att = np.asarray(inputs["rel_att"], np.float32)
    rel_msg = np.asarray(inputs["rel_msg"], np.float32)
    rel_pri = np.asarray(inputs["rel_pri"], np.float32)
    skip = np.asarray(inputs["skip"], np.float32)
    esrc = np.asarray(inputs["edge_src"])
    edst = np.asarray(inputs["edge_dst"])

    alpha = float(1.0 / (1.0 + np.exp(-skip[0])))

    # k weights fp8 (DoubleRow), v weights bf16, v bias row
    wk8 = Wk.T.reshape(2, 128, D).transpose(1, 0, 2).astype(F8N).copy()
    wv16 = Wv.T.reshape(2, 128, D).transpose(1, 0, 2).astype(BF).copy()
    bvv = bv[None, :].astype(BF)

    # qa fold
    WqT4 = Wq.T.reshape(D, H, DK)
    As = rel_att * (rel_pri[:, :, None, None] / SQRT_DK)
    Gq = np.einsum("ihf,rhdf->rihd", WqT4, As).reshape(R, D, D)
    bqa_full = np.einsum("hf,rhdf->rhd", bq.reshape(H, DK), As).reshape(R, D)
    wqa = np.stack([
        np.concatenate([Gq[2 * p], Gq[2 * p + 1]], axis=1).reshape(2, 128, 512)
        for p in range(2)]).transpose(2, 0, 1, 3).astype(F8N).copy()  # [128, pr, ks, 512]
    bqa = np.stack([
        np.concatenate([bqa_full[2 * p], bqa_full[2 * p + 1]])
        for p in range(2)])[None, :, :].astype(BF)                    # [1, pr, 512]

    # output transform fold (alpha included)
    Wa4 = Wa.reshape(D, H, DK)
    wt = (alpha * np.einsum("rhdf,ohf->rhdo", rel_msg, Wa4) / R).reshape(R, 2, 128, D)
    wt = wt.transpose(2, 0, 1, 3).astype(BF).copy()                   # [128, R, ks, 256]

    ident = np.eye(128, dtype=BF)

    common = dict(wk8=wk8, wv16=wv16, bvv=bvv, wqa=wqa, bqa=bqa, wt=wt,
                  ident=ident)

    # per-core uniq determines NGP (must be static across cores)
    cores = []
    for c in range(NC_):
        srcs = []
        for r in range(R):
            sel = (edst[r] >= c * NLOC) & (edst[r] < (c + 1) * NLOC)
            srcs.append(esrc[r][sel])
        uniq = np.unique(np.concatenate(srcs))
        cores.append(uniq)
    NGP = max(len(u) for u in cores)
    NGP = ((NGP + 2047) // 2048) * 2048

    in_maps = []
    cmax = np.zeros(NPAIR, np.int64)
    prepped = []
    for c in range(NC_):
        uniq = cores[c]
        okv = np.zeros((128, R, NSUB), np.int32)
        S = np.zeros((128, R, NSUB, 128), np.float32)
        ST = np.zeros((128, R, NSUB, 128), np.float32)
        for r in range(R):
            okv[:, r], S[:, r], ST[:, r] = _pack_edges(esrc[r], edst[r], c, uniq)
        # first-use pair per table row; stable-reorder rows by it
        first_use = np.full(len(uniq), NPAIR - 1, np.int64)
        for P in range(NPAIR - 1, -1, -1):
            hi = min(3 * P + 3, NSUB)
            rows = okv[:, :, 3 * P:hi].ravel()
            first_use[rows] = P
        order = np.argsort(first_use, kind="stable")
        inv = np.empty_like(order)
        inv[order] = np.arange(len(order))
        okv = inv[okv].astype(np.int32)
        uniq = uniq[order]
        cnt = np.bincount(first_use, minlength=NPAIR)
        cmax = np.maximum(cmax, np.cumsum(cnt))
        prepped.append((uniq, okv, S, ST))
    CPREF = [int(v) for v in cmax]
    for c in range(NC_):
        uniq, okv, S, ST = prepped[c]
        xgT = np.zeros((D, NGP), BF)
        xgT[:, :len(uniq)] = x[uniq].T.astype(BF)
        xgT8 = np.zeros((D, NGP), F8N)
        xgT8[:, :len(uniq)] = x[uniq].T.astype(F8N)
        xqT8 = np.zeros((D, NQ), F8N)
        xqT8[:, :NLOC] = x[c * NLOC:(c + 1) * NLOC].T.astype(F8N)
        xb = np.zeros((NQ, D), BF)
        xb[:NLOC] = ((1.0 - alpha) * x[c * NLOC:(c + 1) * NLOC]
                     + alpha * ba).astype(BF)
        # pair-major S/ST split into 4 full-128 pieces:
        # j0=sub0(ch0), j1=sub1 rows[0:64) (ch0), j2=sub1 rows[64:128) (ch1),
        # j3=sub2(ch1).  [NPAIR, 128, R, 4, 128]
        sm = np.zeros((NPAIR, 128, R, 4, 128), F8N)
        st = np.zeros((NPAIR, 128, R, 4, 128), F8N)
        for P in range(NPAIR):
            s0, s1, s2 = 3 * P, 3 * P + 1, 3 * P + 2
            sm[P, :, :, 0] = S[:, :, s0].astype(F8N)
            sm[P, 0:64, :, 1] = S[0:64, :, s1].astype(F8N)
            sm[P, 64:128, :, 2] = S[64:128, :, s1].astype(F8N)
            if s2 < NSUB:
                sm[P, :, :, 3] = S[:, :, s2].astype(F8N)
            st[P, :, :, 0] = ST[:, :, s0].astype(F8N)
            st[P, :, :, 1] = (ST[:, :, s1] * (np.arange(128) < 64)).astype(F8N)
            st[P, :, :, 2] = (ST[:, :, s1] * (np.arange(128) >= 64)).astype(F8N)
            if s2 < NSUB:
                st[P, :, :, 3] = ST[:, :, s2].astype(F8N)
        # dma_gather int16 indices: per pair, linear order i = (r*3+s)*128 + p,
        # wrapped [16, 96] (idx i at [i%16, i//16]) then replicated to 128 rows
        okv16 = np.zeros((NPAIR, 128, 96), np.int16)
        for P in range(NPAIR):
            linear = okv[:, :, 3 * P:3 * P + 3].transpose(1, 2, 0).reshape(1536)
            blk = linear.reshape(96, 16).T.astype(np.int16)     # [16, 96]
            okv16[P] = np.tile(blk, (8, 1))
        in_maps.append(dict(common, xgT=xgT, xgT8=xgT8, xqT8=xqT8, xb=xb,
                            okv=okv, okv16=okv16, smat=sm, stmat=st))
    return in_maps, (NGP, tuple(CPREF))


# ---------------------------------------------------------------- device build
def _build_nc(sig):
    NGP, CPREF = sig
    nc = bacc.Bacc("TRN2", target_bir_lowering=False, debug=False, num_devices=NC_)
    dt = nc.dram_tensor
    xgT_in = dt("xgT", [D, NGP], BF16, kind="ExternalInput").ap()
    xgT8_in = dt("xgT8", [D, NGP], F8, kind="ExternalInput").ap()
    xqT8_in = dt("xqT8", [D, NQ], F8, kind="ExternalInput").ap()
    xb_in = dt("xb", [NQ, D], BF16, kind="ExternalInput").ap()
    wk8_in = dt("wk8", [128, 2, D], F8, kind="ExternalInput").ap()
    wv16_in = dt("wv16", [128, 2, D], BF16, kind="ExternalInput").ap()
    bvv_in = dt("bvv", [1, D], BF16, kind="ExternalInput").ap()
    wqa_in = dt("wqa", [128, 2, 2, 512], F8, kind="ExternalInput").ap()
    bqa_in = dt("bqa", [1, 2, 512], BF16, kind="ExternalInput").ap()
    wt_in = dt("wt", [128, R, 2, D], BF16, kind="ExternalInput").ap()
    ident_in = dt("ident", [128, 128], BF16, kind="ExternalInput").ap()
    okv_in = dt("okv", [128, R, NSUB], I32, kind="ExternalInput").ap()
    okv16_in = dt("okv16", [NPAIR, 16, 96], mybir.dt.int16,
                  kind="ExternalInput").ap()
    smat_in = dt("smat", [NPAIR, 128, R, 4, 128], F8, kind="ExternalInput").ap()
    stmat_in = dt("stmat", [NPAIR, 128, R, 4, 128], F8, kind="ExternalInput").ap()
    out = dt("out", [NQ, D], F32, kind="ExternalOutput").ap()

    kvt = dt("kvt", [NGP, 768], mybir.dt.uint8, kind="Internal").ap()
    NGROUP = (CPREF[-1] + 511) // 512

    with tile.TileContext(nc) as tc:
        with tc.tile_pool(name="const", bufs=1) as cp:
            wk8_t = cp.tile([128, 2, D], F8)
            nc.sync.dma_start(wk8_t[:], wk8_in[:])
            wv16_t = cp.tile([128, 2, D], BF16)
            nc.sync.dma_start(wv16_t[:], wv16_in[:])
            bvv_t = cp.tile([1, D], BF16)
            nc.sync.dma_start(bvv_t[:], bvv_in[:])
            wqa_t = cp.tile([128, 2, 2, 512], F8)
            nc.sync.dma_start(wqa_t[:], wqa_in[:])
            bqa_t = cp.tile([1, 2, 512], BF16)
            nc.sync.dma_start(bqa_t[:], bqa_in[:])
            wt_t = cp.tile([128, R, 2, D], BF16)
            nc.sync.dma_start(wt_t[:], wt_in[:])
            ident_t = cp.tile([128, 128], BF16)
            nc.sync.dma_start(ident_t[:], ident_in[:])
            okv_t = cp.tile([128, R, NSUB], I32)
            nc.sync.dma_start(okv_t[:], okv_in[:])
            okv16_t = cp.tile([16, NPAIR, 96], mybir.dt.int16)
            nc.sync.dma_start(okv16_t[:],
                              okv16_in.rearrange("n p s -> p n s"))
            ones_bf = cp.tile([1, 128], BF16)
            nc.vector.memset(ones_bf[:], 1.0)
            xqT_t = cp.tile([128, 2, NQ], F8)
            nc.sync.dma_start(
                xqT_t[:], xqT8_in.rearrange("(ks p) n -> p ks n", ks=2))

            with (
                tc.tile_pool(name="xload", bufs=2) as xp,
                tc.tile_pool(name="kvsb", bufs=3) as kvp,
                tc.tile_pool(name="sst", bufs=3) as sp,
                tc.tile_pool(name="gath", bufs=2) as gp,
                tc.tile_pool(name="qaws", bufs=2) as qwp,
                tc.tile_pool(name="edve", bufs=4) as ep,
                tc.tile_pool(name="zts", bufs=6) as zp,
                tc.tile_pool(name="fin", bufs=3) as fp,
                tc.tile_pool(name="psQA", bufs=2, space="PSUM") as psQA,
                tc.tile_pool(name="psQB2", bufs=2, space="PSUM") as psQB2,
                tc.tile_pool(name="psNum", bufs=2, space="PSUM") as psNum,
                tc.tile_pool(name="psR", bufs=2, space="PSUM") as psR,
            ):
                gsem = nc.alloc_semaphore("swdge_dma")
                bstate = {"t": 0, "xT": None, "kvs": None}
                NTILE = NGROUP * 4

                def emit_tiles(ntiles):
                    done = 0
                    while done < ntiles and bstate["t"] < NTILE:
                        ti = bstate["t"]
                        g, t = divmod(ti, 4)
                        w, gi = divmod(g, 4)
                        if gi == 0 and t == 0:
                            xT = xp.tile([128, 2, 2048], BF16, tag="xT", name=f"xT{w}")
                            nc.sync.dma_start(
                                xT[:], xgT_in[:, w * 2048:(w + 1) * 2048]
                                .rearrange("(ks p) n -> p ks n", ks=2))
                            xT8 = xp.tile([128, 2, 2048], F8, tag="xT8",
                                          name=f"xT8{w}")
                            nc.sync.dma_start(
                                xT8[:], xgT8_in[:, w * 2048:(w + 1) * 2048]
                                .rearrange("(ks p) n -> p ks n", ks=2))
                            bstate["xT"] = (xT, xT8)
                        if t == 0:
                            bstate["kvs"] = kvp.tile(
                                [128, 4, 768], mybir.dt.uint8, tag="kvs",
                                name=f"kvs{g}")
                        (xT, xT8), kvs = bstate["xT"], bstate["kvs"]
                        nt = gi * 4 + t
                        if t % 2 == 0:
                            bstate["pk"] = psR.tile([128, 512], F32, tag="mt",
                                                    name=f"pk{g}_{t}")
                            bstate["pv"] = psR.tile([128, 512], F32, tag="mt",
                                                    name=f"pv{g}_{t}")
                        pk, pv = bstate["pk"], bstate["pv"]
                        half = (t % 2) * 256
                        nc.tensor.matmul(
                            pk[:, half:half + 256], xT8[:, :, nt * 128:(nt + 1) * 128],
                            wk8_t[:], start=True, stop=True,
                            perf_mode=mybir.MatmulPerfMode.DoubleRow)
                        for ks in range(2):
                            nc.tensor.matmul(
                                pv[:, half:half + 256],
                                xT[:, ks, nt * 128:(nt + 1) * 128],
                                wv16_t[:, ks], start=(ks == 0), stop=False)
                        nc.tensor.matmul(pv[:, half:half + 256], ones_bf[:],
                                         bvv_t[:], start=False, stop=True)
                        if t % 2 == 1:
                            nc.scalar.copy(
                                kvs[:, t - 1:t + 1, 0:256].bitcast(F8),
                                pk[:].rearrange("p (s f) -> p s f", s=2))
                            nc.scalar.copy(
                                kvs[:, t - 1:t + 1, 256:768].bitcast(BF16),
                                pv[:].rearrange("p (s f) -> p s f", s=2))
                        if t == 3:
                            base = g * 512
                            nc.sync.dma_start(
                                kvt[base:base + 512].rearrange("(s p) f -> p s f", s=4),
                                kvs[:])
                        bstate["t"] += 1
                        done += 1

                def emit_groups(upto):
                    need = min(upto, NGROUP) * 4
                    if bstate["t"] < need:
                        emit_tiles(need - bstate["t"])

                GATHER_MODE = "split"

                def fetch_kvg(P):
                    t = gp.tile([128, R, 3, 768], mybir.dt.uint8, tag="kvg",
                                name=f"kvg{P}")
                    if GATHER_MODE == "swdge":
                        nc.gpsimd.dma_gather(
                            out_ap=t.rearrange("p r s f -> p (r s) f"),
                            in_ap=kvt[0:CPREF[P]],
                            idxs_ap=okv16_t[:, P, :],
                            num_idxs=1536, num_idxs_reg=1536, elem_size=768,
                            single_packet=False)
                    elif GATHER_MODE == "batched":
                        nc.gpsimd.indirect_dma_start(
                            out=t.rearrange("p r s f -> p (r s) f"), out_offset=None,
                            in_=kvt[0:CPREF[P]],
                            in_offset=bass.IndirectOffsetOnAxis(
                                ap=okv_t[:, :, 3 * P:3 * P + 3], axis=0))
                    elif GATHER_MODE == "perrel":
                        ns_ = 2 if P == NPAIR - 1 else 3
                        for r_ in range(R):
                            nc.gpsimd.indirect_dma_start(
                                out=t[:, r_, 0:ns_, :], out_offset=None,
                                in_=kvt[0:CPREF[P]],
                                in_offset=bass.IndirectOffsetOnAxis(
                                    ap=okv_t[:, r_, 3 * P:3 * P + ns_], axis=0))
                    else:
                        ns_ = 2 if P == NPAIR - 1 else 3
                        for r_ in range(R):
                            for s_ in range(ns_):
                                nc.gpsimd.indirect_dma_start(
                                    out=t[:, r_, s_, :], out_offset=None,
                                    in_=kvt[0:CPREF[P]],
                                    in_offset=bass.IndirectOffsetOnAxis(
                                        ap=okv_t[:, r_, 3 * P + s_:3 * P + s_ + 1],
                                        axis=0))
                    return t

                bstate["kvg"] = None
                for P in range(NPAIR):
                    last = (P == NPAIR - 1)
                    ns = 2 if last else 3
                    nch = 1 if last else 2
                    emit_groups((CPREF[min(P + 2, NPAIR - 1)] + 511) // 512)
                    if last:
                        emit_groups(NGROUP)
                    if bstate["kvg"] is None:
                        bstate["kvg"] = fetch_kvg(P)
                    kvg = bstate["kvg"]
                    bstate["kvg"] = fetch_kvg(P + 1) if not last else None
                    # qa window build: qaw [128 n, ch, (pr, 512)]
                    qaw = qwp.tile([128, 2, 1024], BF16, tag="qaw")
                    for ch in range(nch):
                        nb = P * 256 + ch * 128
                        for pr in range(2):
                            qab = psR.tile([128, 512], F32, tag="mt",
                                           name=f"qab{P}_{ch}_{pr}")
                            nc.tensor.matmul(
                                qab[:], xqT_t[:, :, nb:nb + 128],
                                wqa_t[:, pr], start=True, stop=False,
                                perf_mode=mybir.MatmulPerfMode.DoubleRow)
                            nc.tensor.matmul(qab[:], ones_bf[:], bqa_t[:, pr],
                                             start=False, stop=True)
                            nc.scalar.copy(qaw[:, ch, pr * 512:(pr + 1) * 512], qab[:])
                    S_t = sp.tile([128, R, 4, 128], F8, tag="S")
                    nc.sync.dma_start(S_t[:], smat_in[P])
                    ST_t = sp.tile([128, R, 4, 128], F8, tag="ST")
                    nc.sync.dma_start(ST_t[:], stmat_in[P])

                    zts = []

                    def stageA(r):
                        qlo = (r // 2) * 512 + (r % 2) * 256

                        def qwsl(ch, lo=qlo):
                            return qaw[:, ch, lo:lo + 256]
                        kv_g = kvg[:, r]
                        qa01 = psQA.tile([128, 512], F32, tag="qa01", name=f"qa01_{P}_{r}")
                        qa2d = psQB2.tile([128, 512], F32, tag="qa2d", name=f"qa2d_{P}_{r}")
                        nc.tensor.matmul(qa01[:, 0:256], ST_t[:, r, 0, :], qwsl(0),
                                         start=True, stop=True)
                        nc.tensor.matmul(qa01[:, 256:512], ST_t[:, r, 1, :],
                                         qwsl(0), start=True, stop=False)
                        nc.tensor.matmul(qa01[:, 256:512], ST_t[:, r, 2, :],
                                         qwsl(1 if not last else 0),
                                         start=False, stop=True)
                        if not last:
                            nc.tensor.matmul(qa2d[:, 0:256], ST_t[:, r, 3, :], qwsl(1),
                                             start=True, stop=True)
                        return kv_g, qa01, qa2d

                    def stageB(r, kv_g, qa01, qa2d):
                        prodb = ep.tile([128, 3, 256], BF16, tag="prodb")
                        nc.vector.tensor_tensor(
                            out=prodb[:, :2], in0=kv_g[:, :2, 0:256].bitcast(F8),
                            in1=qa01[:].rearrange("p (s f) -> p s f", f=256),
                            op=Alu.mult)
                        if not last:
                            nc.vector.tensor_tensor(
                                out=prodb[:, 2], in0=kv_g[:, 2, 0:256].bitcast(F8),
                                in1=qa2d[:, 0:256], op=Alu.mult)
                        pr4 = prodb.rearrange("p s (h d) -> p s h d", h=4)
                        fold = ep.tile([128, 3, 4, 32], BF16, tag="fold")
                        nc.vector.tensor_tensor(
                            out=fold[:, :ns], in0=pr4[:, :ns, :, 0:32],
                            in1=pr4[:, :ns, :, 32:64], op=Alu.add)
                        attf = ep.tile([128, 3, 4], F32, tag="attf")
                        nc.vector.tensor_reduce(
                            attf[:, :ns], fold[:, :ns],
                            axis=mybir.AxisListType.X, op=Alu.add)
                        pb16 = ep.tile([128, 3, 4, 16], BF16, tag="pb16")
                        nc.scalar.activation(
                            pb16[:, :ns],
                            attf[:, :ns, :, None].to_broadcast([128, ns, 4, 16]),
                            Act.Exp)
                        # den (node-major) then per-edge 1/den expansion
                        nc.tensor.matmul(qa2d[:, 256:260], S_t[:, r, 0, :],
                                         pb16[:, 0, :, 0], start=True, stop=False)
                        nc.tensor.matmul(qa2d[:, 256:260], S_t[:, r, 1, :],
                                         pb16[:, 1, :, 0], start=False, stop=True)
                        if not last:
                            nc.tensor.matmul(qa2d[:, 260:264], S_t[:, r, 2, :],
                                             pb16[:, 1, :, 0], start=True, stop=False)
                            nc.tensor.matmul(qa2d[:, 260:264], S_t[:, r, 3, :],
                                             pb16[:, 2, :, 0], start=False, stop=True)
                        rdenf = ep.tile([128, 8], F32, tag="rdenf")
                        nc.vector.tensor_scalar_max(rdenf[:, :nch * 4],
                                                    qa2d[:, 256:256 + nch * 4], 1e-9)
                        rden = ep.tile([128, 8], BF16, tag="rden")
                        with nc.allow_low_precision("1/den in bf16 is within tol"):
                            nc.vector.reciprocal(rden[:, :nch * 4],
                                                 rdenf[:, :nch * 4])
                        rdE = qa2d[:, 272:284].rearrange("p (s h) -> p s h", s=3)
                        nc.tensor.matmul(rdE[:, 0], ST_t[:, r, 0, :], rden[:, 0:4],
                                         start=True, stop=True)
                        nc.tensor.matmul(rdE[:, 1], ST_t[:, r, 1, :], rden[:, 0:4],
                                         start=True, stop=last)
                        if not last:
                            nc.tensor.matmul(rdE[:, 1], ST_t[:, r, 2, :],
                                             rden[:, 4:8], start=False, stop=True)
                            nc.tensor.matmul(rdE[:, 2], ST_t[:, r, 3, :],
                                             rden[:, 4:8], start=True, stop=True)
                        W = ep.tile([128, 3, 4, 16], BF16, tag="W")
                        nc.vector.tensor_tensor(
                            out=W[:, :ns], in0=pb16[:, :ns],
                            in1=rdE[:, :ns, :, None].to_broadcast([128, ns, 4, 16]),
                            op=Alu.mult)
                        Y = ep.tile([128, 3, 256], BF16, tag="Y")
                        nc.vector.tensor_tensor(
                            out=Y[:, :ns].rearrange(
                                "p s (h o e) -> p s h o e", h=4, e=16),
                            in0=kv_g[:, :ns, 256:768].bitcast(BF16).rearrange(
                                "p s (h o e) -> p s h o e", h=4, e=16),
                            in1=W[:, :ns, :, None, :]
                                .to_broadcast([128, ns, 4, 4, 16]),
                            op=Alu.mult)

                        # hd-major weighted segment sum: numT[hd, u]
                        numT = psNum.tile([128, 2, 2, 128], F32, tag="num")
                        for half in range(2):
                            ysl = [Y[:, s_, half * 128:(half + 1) * 128]
                                   for s_ in range(3)]
                            nc.tensor.matmul(numT[:, half, 0], ysl[0],
                                             S_t[:, r, 0, :], start=True, stop=False)
                            nc.tensor.matmul(numT[:, half, 0], ysl[1],
                                             S_t[:, r, 1, :], start=False, stop=True)
                            if not last:
                                nc.tensor.matmul(numT[:, half, 1], ysl[1],
                                                 S_t[:, r, 2, :], start=True,
                                                 stop=False)
                                nc.tensor.matmul(numT[:, half, 1], ysl[2],
                                                 S_t[:, r, 3, :], start=False,
                                                 stop=True)
                        zS = zp.tile([128, 2, 2, 128], BF16, tag="ztS")
                        nc.scalar.copy(zS[:, :, :nch], numT[:, :, :nch])
                        zts.append(zS)
                        emit_tiles(2 if P < 10 else 1)

                    for r in range(R):
                        stageB(r, *stageA(r))

                    # transform + blend (xb added via identity matmul)
                    xbt = fp.tile([128, 2, 256], BF16, tag="xbt")
                    nc.sync.dma_start(
                        xbt[:, :nch],
                        xb_in[P * 256:P * 256 + nch * 128]
                        .rearrange("(c p) f -> p c f", c=nch))
                    pt = psR.tile([128, 512], F32, tag="mt", name=f"pt{P}")
                    for ch in range(nch):
                        for i in range(8):
                            r, ks = i // 2, i % 2
                            nc.tensor.matmul(
                                pt[:, ch * 256:(ch + 1) * 256],
                                zts[r][:, ks, ch],
                                wt_t[:, r, ks], start=(i == 0), stop=False)
                        nc.tensor.matmul(
                            pt[:, ch * 256:(ch + 1) * 256], ident_t[:],
                            xbt[:, ch], start=False, stop=True)
                    o = fp.tile([128, 2, 256], F32, tag="o")
                    nc.scalar.copy(o[:, :nch], pt[:, :nch * 256]
                                   .rearrange("p (c f) -> p c f", f=256))
                    nc.sync.dma_start(
                        out[P * 256:P * 256 + nch * 128]
                        .rearrange("(c p) f -> p c f", c=nch),
                        o[:, :nch])
    nc.compile()
    return nc


def kernel(**inputs):
    in_maps, sig = _host_prep_cached(inputs)
    key = ("nc", sig)
    if key not in _cache:
        _cache[key] = _build_nc(sig)
        _cache["nc"] = _cache[key]
    nc = _cache[key]
    res = run_bass_kernel_spmd(nc, in_maps, core_ids=list(range(NC_)))
    return np.concatenate(
        [res.results[c]["out"][:NLOC] for c in range(NC_)], axis=0)


def _host_prep_cached(inputs):
    return _host_prep(inputs)



# revision 34
# speedup vs baseline: 1.0010x; 1.0010x over previous
"""HGT layer kernel for 8 Trainium2 NeuronCores (Bass/Tile) — v2.

Design (vs. v1 baseline):
- dst-range sharding: core c owns dst nodes [c*6250, (c+1)*6250); edges bucketed
  per dst-owner, chunk-aligned packing (CAP=192 per 128-node chunk, NSUB=74
  subtiles per relation) so the SPMD program is static across cores.
- Compact kv table: only the ~20k unique src nodes each core actually
  references (host computes uniq + remap); k-half has NO bias (cancels in
  segment softmax), v-half keeps bv (so empty segments stay exactly 0).
- Host pre-transposes x (xgT/xqT) so phase A needs no DMA transposes.
- qa side: NO DRAM table and NO gathers — per-pair qA windows are built
  on-the-fly (PE matmul) and expanded per-edge with one-hot S^T matmuls.
- Node-major segment sums: num [node,256] + 4-wide den via the same one-hot S
  lhsT; division with free-dim broadcast; z transposed back via PE transposes
  for the folded output transform (rel_msg x Wa x alpha/R folded into wt).
- exp via Act engine broadcast to 64-wide so Y multiply runs in DVE 2x mode.
- alpha, ba folded host-side into xb_pre = (1-alpha)x + alpha*ba.
"""
import sys, types
import numpy as np
import ml_dtypes

if "antenv.axon_hooks" not in sys.modules:
    try:
        from trn_agent_boot.trn_boot import _ntff_profile_via_ctypes as _mk_hook
        _m = types.ModuleType("antenv.axon_hooks")
        _m.get_axon_ntff_profile_hook = lambda: None
        sys.modules["antenv.axon_hooks"] = _m
    except Exception:
        pass

import concourse.bass as bass
import concourse.bacc as bacc
import concourse.tile as tile
import concourse.mybir as mybir
from concourse.bass_utils import run_bass_kernel_spmd

BF16 = mybir.dt.bfloat16
F32 = mybir.dt.float32
I32 = mybir.dt.int32
BF = ml_dtypes.bfloat16
F8 = mybir.dt.float8e4
F8N = ml_dtypes.float8_e4m3
Alu = mybir.AluOpType
Act = mybir.ActivationFunctionType

N, D, R, H, DK = 50000, 256, 4, 4, 64
NC_ = 8
NLOC = N // NC_          # 6250
CH = 128
NCHUNK = 49
NPAIR = 25
NSUB = 75                # 74 used + 1 pad so every pair has 3
CAP = 192
NQ = 6400                # padded own-node count
SQRT_DK = 8.0

_cache: dict = {}


def pb4(t):
    """[128, 3, 256] -> [128, 3, 4] view picking element 0 of each 64-block."""
    return t.rearrange("p s (h d) -> p s h d", h=4)[:, :, :, 0]



# ---------------------------------------------------------------- host prep
def _pack_edges(src, dst, core, uniq):
    """Chunk-aligned packing (as v1): returns okv [128, NSUB] int32 (compact
    kv rows), S [128, NSUB, 128] bf16, ST [128, NSUB, 128] bf16."""
    sel = (dst >= core * NLOC) & (dst < (core + 1) * NLOC)
    es = src[sel].astype(np.int64)
    ed = (dst[sel] - core * NLOC).astype(np.int64)
    chunk = ed >> 7
    order = np.lexsort((es, chunk))
    es, ed, chunk = es[order], ed[order], chunk[order]
    counts = np.bincount(chunk, minlength=NCHUNK)
    if counts.max() > CAP:
        raise RuntimeError(f"chunk overflow: {counts.max()} > {CAP}")
    starts = np.zeros(NCHUNK, np.int64)
    starts[1:] = np.cumsum(counts)[:-1]
    slot = np.arange(len(ed)) - starts[chunk]
    P = chunk >> 1
    even = (chunk & 1) == 0
    sub = np.where(even,
                   np.where(slot < 128, 3 * P, 3 * P + 1),
                   np.where(slot < 64, 3 * P + 1, 3 * P + 2))
    part = np.where(even,
                    np.where(slot < 128, slot, slot - 128),
                    np.where(slot < 64, 64 + slot, slot - 64))
    okv = np.zeros((128, NSUB), np.int32)
    S = np.zeros((128, NSUB, 128), np.float32)
    kvrow = np.searchsorted(uniq, es).astype(np.int32)
    okv[part, sub] = kvrow
    S[part, sub, ed & 127] = 1.0
    ST = S.transpose(2, 1, 0).copy()    # [node, sub, edge]
    return okv, S, ST


def _host_prep(inputs):
    x = np.asarray(inputs["x"], np.float32)
    Wk = np.asarray(inputs["Wk"], np.float32)
    Wq, bq = np.asarray(inputs["Wq"], np.float32), np.asarray(inputs["bq"], np.float32)
    Wv, bv = np.asarray(inputs["Wv"], np.float32), np.asarray(inputs["bv"], np.float32)
    Wa, ba = np.asarray(inputs["Wa"], np.float32), np.asarray(inputs["ba"], np.float32)
    rel_att = np.asarray(inputs["rel_att"], np.float32)
    rel_msg = np.asarray(inputs["rel_msg"], np.float32)
    rel_pri = np.asarray(inputs["rel_pri"], np.float32)
    skip = np.asarray(inputs["skip"], np.float32)
    esrc = np.asarray(inputs["edge_src"])
    edst = np.asarray(inputs["edge_dst"])

    alpha = float(1.0 / (1.0 + np.exp(-skip[0])))

    # k weights fp8 (DoubleRow), v weights bf16, v bias row
    wk8 = Wk.T.reshape(2, 128, D).transpose(1, 0, 2).astype(F8N).copy()
    wv16 = Wv.T.reshape(2, 128, D).transpose(1, 0, 2).astype(BF).copy()
    bvv = bv[None, :].astype(BF)

    # qa fold
    WqT4 = Wq.T.reshape(D, H, DK)
    As = rel_att * (rel_pri[:, :, None, None] / SQRT_DK)
    Gq = np.einsum("ihf,rhdf->rihd", WqT4, As).reshape(R, D, D)
    bqa_full = np.einsum("hf,rhdf->rhd", bq.reshape(H, DK), As).reshape(R, D)
    wqa = np.stack([
        np.concatenate([Gq[2 * p], Gq[2 * p + 1]], axis=1).reshape(2, 128, 512)
        for p in range(2)]).transpose(2, 0, 1, 3).astype(F8N).copy()  # [128, pr, ks, 512]
    bqa = np.stack([
        np.concatenate([bqa_full[2 * p], bqa_full[2 * p + 1]])
        for p in range(2)])[None, :, :].astype(BF)                    # [1, pr, 512]

    # output transform fold (alpha included)
    Wa4 = Wa.reshape(D, H, DK)
    wt = (alpha * np.einsum("rhdf,ohf->rhdo", rel_msg, Wa4) / R).reshape(R, 2, 128, D)
    wt = wt.transpose(2, 0, 1, 3).astype(BF).copy()                   # [128, R, ks, 256]

    ident = np.eye(128, dtype=BF)

    common = dict(wk8=wk8, wv16=wv16, bvv=bvv, wqa=wqa, bqa=bqa, wt=wt,
                  ident=ident)

    # per-core uniq determines NGP (must be static across cores)
    cores = []
    for c in range(NC_):
        srcs = []
        for r in range(R):
            sel = (edst[r] >= c * NLOC) & (edst[r] < (c + 1) * NLOC)
            srcs.append(esrc[r][sel])
        uniq = np.unique(np.concatenate(srcs))
        cores.append(uniq)
    NGP = max(len(u) for u in cores)
    NGP = ((NGP + 2047) // 2048) * 2048

    in_maps = []
    cmax = np.zeros(NPAIR, np.int64)
    prepped = []
    for c in range(NC_):
        uniq = cores[c]
        okv = np.zeros((128, R, NSUB), np.int32)
        S = np.zeros((128, R, NSUB, 128), np.float32)
        ST = np.zeros((128, R, NSUB, 128), np.float32)
        for r in range(R):
            okv[:, r], S[:, r], ST[:, r] = _pack_edges(esrc[r], edst[r], c, uniq)
        # first-use pair per table row; stable-reorder rows by it
        first_use = np.full(len(uniq), NPAIR - 1, np.int64)
        for P in range(NPAIR - 1, -1, -1):
            hi = min(3 * P + 3, NSUB)
            rows = okv[:, :, 3 * P:hi].ravel()
            first_use[rows] = P
        order = np.argsort(first_use, kind="stable")
        inv = np.empty_like(order)
        inv[order] = np.arange(len(order))
        okv = inv[okv].astype(np.int32)
        uniq = uniq[order]
        cnt = np.bincount(first_use, minlength=NPAIR)
        cmax = np.maximum(cmax, np.cumsum(cnt))
        prepped.append((uniq, okv, S, ST))
    CPREF = [int(v) for v in cmax]
    for c in range(NC_):
        uniq, okv, S, ST = prepped[c]
        xgT = np.zeros((D, NGP), BF)
        xgT[:, :len(uniq)] = x[uniq].T.astype(BF)
        xgT8 = np.zeros((D, NGP), F8N)
        xgT8[:, :len(uniq)] = x[uniq].T.astype(F8N)
        xqT8 = np.zeros((D, NQ), F8N)
        xqT8[:, :NLOC] = x[c * NLOC:(c + 1) * NLOC].T.astype(F8N)
        xb = np.zeros((NQ, D), BF)
        xb[:NLOC] = ((1.0 - alpha) * x[c * NLOC:(c + 1) * NLOC]
                     + alpha * ba).astype(BF)
        # pair-major S/ST split into 4 full-128 pieces:
        # j0=sub0(ch0), j1=sub1 rows[0:64) (ch0), j2=sub1 rows[64:128) (ch1),
        # j3=sub2(ch1).  [NPAIR, 128, R, 4, 128]
        sm = np.zeros((NPAIR, 128, R, 4, 128), F8N)
        st = np.zeros((NPAIR, 128, R, 4, 128), F8N)
        for P in range(NPAIR):
            s0, s1, s2 = 3 * P, 3 * P + 1, 3 * P + 2
            sm[P, :, :, 0] = S[:, :, s0].astype(F8N)
            sm[P, 0:64, :, 1] = S[0:64, :, s1].astype(F8N)
            sm[P, 64:128, :, 2] = S[64:128, :, s1].astype(F8N)
            if s2 < NSUB:
                sm[P, :, :, 3] = S[:, :, s2].astype(F8N)
            st[P, :, :, 0] = ST[:, :, s0].astype(F8N)
            st[P, :, :, 1] = (ST[:, :, s1] * (np.arange(128) < 64)).astype(F8N)
            st[P, :, :, 2] = (ST[:, :, s1] * (np.arange(128) >= 64)).astype(F8N)
            if s2 < NSUB:
                st[P, :, :, 3] = ST[:, :, s2].astype(F8N)
        # dma_gather int16 indices: per pair, linear order i = (r*3+s)*128 + p,
        # wrapped [16, 96] (idx i at [i%16, i//16]) then replicated to 128 rows
        okv16 = np.zeros((NPAIR, 128, 96), np.int16)
        for P in range(NPAIR):
            linear = okv[:, :, 3 * P:3 * P + 3].transpose(1, 2, 0).reshape(1536)
            blk = linear.reshape(96, 16).T.astype(np.int16)     # [16, 96]
            okv16[P] = np.tile(blk, (8, 1))
        in_maps.append(dict(common, xgT=xgT, xgT8=xgT8, xqT8=xqT8, xb=xb,
                            okv=okv, okv16=okv16, smat=sm, stmat=st))
    return in_maps, (NGP, tuple(CPREF))


# ---------------------------------------------------------------- device build
def _build_nc(sig):
    NGP, CPREF = sig
    nc = bacc.Bacc("TRN2", target_bir_lowering=False, debug=False, num_devices=NC_)
    dt = nc.dram_tensor
    xgT_in = dt("xgT", [D, NGP], BF16, kind="ExternalInput").ap()
    xgT8_in = dt("xgT8", [D, NGP], F8, kind="ExternalInput").ap()
    xqT8_in = dt("xqT8", [D, NQ], F8, kind="ExternalInput").ap()
    xb_in = dt("xb", [NQ, D], BF16, kind="ExternalInput").ap()
    wk8_in = dt("wk8", [128, 2, D], F8, kind="ExternalInput").ap()
    wv16_in = dt("wv16", [128, 2, D], BF16, kind="ExternalInput").ap()
    bvv_in = dt("bvv", [1, D], BF16, kind="ExternalInput").ap()
    wqa_in = dt("wqa", [128, 2, 2, 512], F8, kind="ExternalInput").ap()
    bqa_in = dt("bqa", [1, 2, 512], BF16, kind="ExternalInput").ap()
    wt_in = dt("wt", [128, R, 2, D], BF16, kind="ExternalInput").ap()
    ident_in = dt("ident", [128, 128], BF16, kind="ExternalInput").ap()
    okv_in = dt("okv", [128, R, NSUB], I32, kind="ExternalInput").ap()
    okv16_in = dt("okv16", [NPAIR, 16, 96], mybir.dt.int16,
                  kind="ExternalInput").ap()
    smat_in = dt("smat", [NPAIR, 128, R, 4, 128], F8, kind="ExternalInput").ap()
    stmat_in = dt("stmat", [NPAIR, 128, R, 4, 128], F8, kind="ExternalInput").ap()
    out = dt("out", [NQ, D], F32, kind="ExternalOutput").ap()

    kvt = dt("kvt", [NGP, 768], mybir.dt.uint8, kind="Internal").ap()
    NGROUP = (CPREF[-1] + 511) // 512

    with tile.TileContext(nc) as tc:
        with tc.tile_pool(name="const", bufs=1) as cp:
            wk8_t = cp.tile([128, 2, D], F8)
            nc.sync.dma_start(wk8_t[:], wk8_in[:])
            wv16_t = cp.tile([128, 2, D], BF16)
            nc.sync.dma_start(wv16_t[:], wv16_in[:])
            bvv_t = cp.tile([1, D], BF16)
            nc.sync.dma_start(bvv_t[:], bvv_in[:])
            wqa_t = cp.tile([128, 2, 2, 512], F8)
            nc.sync.dma_start(wqa_t[:], wqa_in[:])
            bqa_t = cp.tile([1, 2, 512], BF16)
            nc.sync.dma_start(bqa_t[:], bqa_in[:])
            wt_t = cp.tile([128, R, 2, D], BF16)
            nc.sync.dma_start(wt_t[:], wt_in[:])
            ident_t = cp.tile([128, 128], BF16)
            nc.sync.dma_start(ident_t[:], ident_in[:])
            okv_t = cp.tile([128, R, NSUB], I32)
            nc.sync.dma_start(okv_t[:], okv_in[:])
            okv16_t = cp.tile([16, NPAIR, 96], mybir.dt.int16)
            nc.sync.dma_start(okv16_t[:],
                              okv16_in.rearrange("n p s -> p n s"))
            ones_bf = cp.tile([1, 128], BF16)
            nc.vector.memset(ones_bf[:], 1.0)
            xqT_t = cp.tile([128, 2, NQ], F8)
            nc.sync.dma_start(
                xqT_t[:], xqT8_in.rearrange("(ks p) n -> p ks n", ks=2))

            with (
                tc.tile_pool(name="xload", bufs=2) as xp,
                tc.tile_pool(name="kvsb", bufs=3) as kvp,
                tc.tile_pool(name="sst", bufs=3) as sp,
                tc.tile_pool(name="gath", bufs=3) as gp,
                tc.tile_pool(name="qaws", bufs=2) as qwp,
                tc.tile_pool(name="edve", bufs=4) as ep,
                tc.tile_pool(name="zts", bufs=6) as zp,
                tc.tile_pool(name="fin", bufs=3) as fp,
                tc.tile_pool(name="psQA", bufs=2, space="PSUM") as psQA,
                tc.tile_pool(name="psQB2", bufs=2, space="PSUM") as psQB2,
                tc.tile_pool(name="psNum", bufs=2, space="PSUM") as psNum,
                tc.tile_pool(name="psR", bufs=2, space="PSUM") as psR,
            ):
                gsem = nc.alloc_semaphore("swdge_dma")
                bstate = {"t": 0, "xT": None, "kvs": None}
                NTILE = NGROUP * 4

                def emit_tiles(ntiles):
                    done = 0
                    while done < ntiles and bstate["t"] < NTILE:
                        ti = bstate["t"]
                        g, t = divmod(ti, 4)
                        w, gi = divmod(g, 4)
                        if gi == 0 and t == 0:
                            xT = xp.tile([128, 2, 2048], BF16, tag="xT", name=f"xT{w}")
                            nc.sync.dma_start(
                                xT[:], xgT_in[:, w * 2048:(w + 1) * 2048]
                                .rearrange("(ks p) n -> p ks n", ks=2))
                            xT8 = xp.tile([128, 2, 2048], F8, tag="xT8",
                                          name=f"xT8{w}")
                            nc.sync.dma_start(
                                xT8[:], xgT8_in[:, w * 2048:(w + 1) * 2048]
                                .rearrange("(ks p) n -> p ks n", ks=2))
                            bstate["xT"] = (xT, xT8)
                        if t == 0:
                            bstate["kvs"] = kvp.tile(
                                [128, 4, 768], mybir.dt.uint8, tag="kvs",
                                name=f"kvs{g}")
                        (xT, xT8), kvs = bstate["xT"], bstate["kvs"]
                        nt = gi * 4 + t
                        if t % 2 == 0:
                            bstate["pk"] = psR.tile([128, 512], F32, tag="mt",
                                                    name=f"pk{g}_{t}")
                            bstate["pv"] = psR.tile([128, 512], F32, tag="mt",
                                                    name=f"pv{g}_{t}")
                        pk, pv = bstate["pk"], bstate["pv"]
                        half = (t % 2) * 256
                        nc.tensor.matmul(
                            pk[:, half:half + 256], xT8[:, :, nt * 128:(nt + 1) * 128],
                            wk8_t[:], start=True, stop=True,
                            perf_mode=mybir.MatmulPerfMode.DoubleRow)
                        for ks in range(2):
                            nc.tensor.matmul(
                                pv[:, half:half + 256],
                                xT[:, ks, nt * 128:(nt + 1) * 128],
                                wv16_t[:, ks], start=(ks == 0), stop=False)
                        nc.tensor.matmul(pv[:, half:half + 256], ones_bf[:],
                                         bvv_t[:], start=False, stop=True)
                        if t % 2 == 1:
                            nc.scalar.copy(
                                kvs[:, t - 1:t + 1, 0:256].bitcast(F8),
                                pk[:].rearrange("p (s f) -> p s f", s=2))
                            nc.scalar.copy(
                                kvs[:, t - 1:t + 1, 256:768].bitcast(BF16),
                                pv[:].rearrange("p (s f) -> p s f", s=2))
                        if t == 3:
                            base = g * 512
                            nc.sync.dma_start(
                                kvt[base:base + 512].rearrange("(s p) f -> p s f", s=4),
                                kvs[:])
                        bstate["t"] += 1
                        done += 1

                def emit_groups(upto):
                    need = min(upto, NGROUP) * 4
                    if bstate["t"] < need:
                        emit_tiles(need - bstate["t"])

                GATHER_MODE = "split"

                def fetch_kvg(P):
                    t = gp.tile([128, R, 3, 768], mybir.dt.uint8, tag="kvg",
                                name=f"kvg{P}")
                    if GATHER_MODE == "swdge":
                        nc.gpsimd.dma_gather(
                            out_ap=t.rearrange("p r s f -> p (r s) f"),
                            in_ap=kvt[0:CPREF[P]],
                            idxs_ap=okv16_t[:, P, :],
                            num_idxs=1536, num_idxs_reg=1536, elem_size=768,
                            single_packet=False)
                    elif GATHER_MODE == "batched":
                        nc.gpsimd.indirect_dma_start(
                            out=t.rearrange("p r s f -> p (r s) f"), out_offset=None,
                            in_=kvt[0:CPREF[P]],
                            in_offset=bass.IndirectOffsetOnAxis(
                                ap=okv_t[:, :, 3 * P:3 * P + 3], axis=0))
                    elif GATHER_MODE == "perrel":
                        ns_ = 2 if P == NPAIR - 1 else 3
                        for r_ in range(R):
                            nc.gpsimd.indirect_dma_start(
                                out=t[:, r_, 0:ns_, :], out_offset=None,
                                in_=kvt[0:CPREF[P]],
                                in_offset=bass.IndirectOffsetOnAxis(
                                    ap=okv_t[:, r_, 3 * P:3 * P + ns_], axis=0))
                    else:
                        ns_ = 2 if P == NPAIR - 1 else 3
                        for r_ in range(R):
                            for s_ in range(ns_):
                                nc.gpsimd.indirect_dma_start(
                                    out=t[:, r_, s_, :], out_offset=None,
                                    in_=kvt[0:CPREF[P]],
                                    in_offset=bass.IndirectOffsetOnAxis(
                                        ap=okv_t[:, r_, 3 * P + s_:3 * P + s_ + 1],
                                        axis=0))
                    return t

                bstate["kvq"] = []
                for P in range(NPAIR):
                    last = (P == NPAIR - 1)
                    ns = 2 if last else 3
                    nch = 1 if last else 2
                    emit_groups((CPREF[min(P + 3, NPAIR - 1)] + 511) // 512)
                    if last:
                        emit_groups(NGROUP)
                    while len(bstate["kvq"]) < min(3, NPAIR - P):
                        bstate["kvq"].append(fetch_kvg(P + len(bstate["kvq"])))
                    kvg = bstate["kvq"].pop(0)
                    # qa window build: qaw [128 n, ch, (pr, 512)]
                    qaw = qwp.tile([128, 2, 1024], BF16, tag="qaw")
                    for ch in range(nch):
                        nb = P * 256 + ch * 128
                        for pr in range(2):
                            qab = psR.tile([128, 512], F32, tag="mt",
                                           name=f"qab{P}_{ch}_{pr}")
                            nc.tensor.matmul(
                                qab[:], xqT_t[:, :, nb:nb + 128],
                                wqa_t[:, pr], start=True, stop=False,
                                perf_mode=mybir.MatmulPerfMode.DoubleRow)
                            nc.tensor.matmul(qab[:], ones_bf[:], bqa_t[:, pr],
                                             start=False, stop=True)
                            nc.scalar.copy(qaw[:, ch, pr * 512:(pr + 1) * 512], qab[:])
                    S_t = sp.tile([128, R, 4, 128], F8, tag="S")
                    nc.sync.dma_start(S_t[:], smat_in[P])
                    ST_t = sp.tile([128, R, 4, 128], F8, tag="ST")
                    nc.sync.dma_start(ST_t[:], stmat_in[P])

                    zts = []

                    def stageA(r):
                        qlo = (r // 2) * 512 + (r % 2) * 256

                        def qwsl(ch, lo=qlo):
                            return qaw[:, ch, lo:lo + 256]
                        kv_g = kvg[:, r]
                        qa01 = psQA.tile([128, 512], F32, tag="qa01", name=f"qa01_{P}_{r}")
                        qa2d = psQB2.tile([128, 512], F32, tag="qa2d", name=f"qa2d_{P}_{r}")
                        nc.tensor.matmul(qa01[:, 0:256], ST_t[:, r, 0, :], qwsl(0),
                                         start=True, stop=True)
                        nc.tensor.matmul(qa01[:, 256:512], ST_t[:, r, 1, :],
                                         qwsl(0), start=True, stop=False)
                        nc.tensor.matmul(qa01[:, 256:512], ST_t[:, r, 2, :],
                                         qwsl(1 if not last else 0),
                                         start=False, stop=True)
                        if not last:
                            nc.tensor.matmul(qa2d[:, 0:256], ST_t[:, r, 3, :], qwsl(1),
                                             start=True, stop=True)
                        return kv_g, qa01, qa2d

                    def stageB(r, kv_g, qa01, qa2d):
                        prodb = ep.tile([128, 3, 256], BF16, tag="prodb")
                        nc.vector.tensor_tensor(
                            out=prodb[:, :2], in0=kv_g[:, :2, 0:256].bitcast(F8),
                            in1=qa01[:].rearrange("p (s f) -> p s f", f=256),
                            op=Alu.mult)
                        if not last:
                            nc.vector.tensor_tensor(
                                out=prodb[:, 2], in0=kv_g[:, 2, 0:256].bitcast(F8),
                                in1=qa2d[:, 0:256], op=Alu.mult)
                        pr4 = prodb.rearrange("p s (h d) -> p s h d", h=4)
                        fold = ep.tile([128, 3, 4, 32], BF16, tag="fold")
                        nc.vector.tensor_tensor(
                            out=fold[:, :ns], in0=pr4[:, :ns, :, 0:32],
                            in1=pr4[:, :ns, :, 32:64], op=Alu.add)
                        attf = ep.tile([128, 3, 4], F32, tag="attf")
                        nc.vector.tensor_reduce(
                            attf[:, :ns], fold[:, :ns],
                            axis=mybir.AxisListType.X, op=Alu.add)
                        pb16 = ep.tile([128, 3, 4, 16], BF16, tag="pb16")
                        nc.scalar.activation(
                            pb16[:, :ns],
                            attf[:, :ns, :, None].to_broadcast([128, ns, 4, 16]),
                            Act.Exp)
                        # den (node-major) then per-edge 1/den expansion
                        nc.tensor.matmul(qa2d[:, 256:260], S_t[:, r, 0, :],
                                         pb16[:, 0, :, 0], start=True, stop=False)
                        nc.tensor.matmul(qa2d[:, 256:260], S_t[:, r, 1, :],
                                         pb16[:, 1, :, 0], start=False, stop=True)
                        if not last:
                            nc.tensor.matmul(qa2d[:, 260:264], S_t[:, r, 2, :],
                                             pb16[:, 1, :, 0], start=True, stop=False)
                            nc.tensor.matmul(qa2d[:, 260:264], S_t[:, r, 3, :],
                                             pb16[:, 2, :, 0], start=False, stop=True)
                        rdenf = ep.tile([128, 8], F32, tag="rdenf")
                        nc.vector.tensor_scalar_max(rdenf[:, :nch * 4],
                                                    qa2d[:, 256:256 + nch * 4], 1e-9)
                        rden = ep.tile([128, 8], BF16, tag="rden")
                        with nc.allow_low_precision("1/den in bf16 is within tol"):
                            nc.vector.reciprocal(rden[:, :nch * 4],
                                                 rdenf[:, :nch * 4])
                        rdE = qa2d[:, 272:284].rearrange("p (s h) -> p s h", s=3)
                        nc.tensor.matmul(rdE[:, 0], ST_t[:, r, 0, :], rden[:, 0:4],
                                         start=True, stop=True)
                        nc.tensor.matmul(rdE[:, 1], ST_t[:, r, 1, :], rden[:, 0:4],
                                         start=True, stop=last)
                        if not last:
                            nc.tensor.matmul(rdE[:, 1], ST_t[:, r, 2, :],
                                             rden[:, 4:8], start=False, stop=True)
                            nc.tensor.matmul(rdE[:, 2], ST_t[:, r, 3, :],
                                             rden[:, 4:8], start=True, stop=True)
                        W = ep.tile([128, 3, 4, 16], BF16, tag="W")
                        nc.vector.tensor_tensor(
                            out=W[:, :ns], in0=pb16[:, :ns],
                            in1=rdE[:, :ns, :, None].to_broadcast([128, ns, 4, 16]),
                            op=Alu.mult)
                        Y = ep.tile([128, 3, 256], BF16, tag="Y")
                        nc.vector.tensor_tensor(
                            out=Y[:, :ns].rearrange(
                                "p s (h o e) -> p s h o e", h=4, e=16),
                            in0=kv_g[:, :ns, 256:768].bitcast(BF16).rearrange(
                                "p s (h o e) -> p s h o e", h=4, e=16),
                            in1=W[:, :ns, :, None, :]
                                .to_broadcast([128, ns, 4, 4, 16]),
                            op=Alu.mult)

                        # hd-major weighted segment sum: numT[hd, u]
                        numT = psNum.tile([128, 2, 2, 128], F32, tag="num")
                        for half in range(2):
                            ysl = [Y[:, s_, half * 128:(half + 1) * 128]
                                   for s_ in range(3)]
                            nc.tensor.matmul(numT[:, half, 0], ysl[0],
                                             S_t[:, r, 0, :], start=True, stop=False)
                            nc.tensor.matmul(numT[:, half, 0], ysl[1],
                                             S_t[:, r, 1, :], start=False, stop=True)
                            if not last:
                                nc.tensor.matmul(numT[:, half, 1], ysl[1],
                                                 S_t[:, r, 2, :], start=True,
                                                 stop=False)
                                nc.tensor.matmul(numT[:, half, 1], ysl[2],
                                                 S_t[:, r, 3, :], start=False,
                                                 stop=True)
                        zS = zp.tile([128, 2, 2, 128], BF16, tag="ztS")
                        nc.scalar.copy(zS[:, :, :nch], numT[:, :, :nch])
                        zts.append(zS)
                        emit_tiles(2 if P < 10 else 1)

                    for r in range(R):
                        stageB(r, *stageA(r))

                    # transform + blend (xb added via identity matmul)
                    xbt = fp.tile([128, 2, 256], BF16, tag="xbt")
                    nc.sync.dma_start(
                        xbt[:, :nch],
                        xb_in[P * 256:P * 256 + nch * 128]
                        .rearrange("(c p) f -> p c f", c=nch))
                    pt = psR.tile([128, 512], F32, tag="mt", name=f"pt{P}")
                    for ch in range(nch):
                        for i in range(8):
                            r, ks = i // 2, i % 2
                            nc.tensor.matmul(
                                pt[:, ch * 256:(ch + 1) * 256],
                                zts[r][:, ks, ch],
                                wt_t[:, r, ks], start=(i == 0), stop=False)
                        nc.tensor.matmul(
                            pt[:, ch * 256:(ch + 1) * 256], ident_t[:],
                            xbt[:, ch], start=False, stop=True)
                    o = fp.tile([128, 2, 256], F32, tag="o")
                    nc.scalar.copy(o[:, :nch], pt[:, :nch * 256]
                                   .rearrange("p (c f) -> p c f", f=256))
                    nc.sync.dma_start(
                        out[P * 256:P * 256 + nch * 128]
                        .rearrange("(c p) f -> p c f", c=nch),
                        o[:, :nch])
    nc.compile()
    return nc


def kernel(**inputs):
    in_maps, sig = _host_prep_cached(inputs)
    key = ("nc", sig)
    if key not in _cache:
        _cache[key] = _build_nc(sig)
        _cache["nc"] = _cache[key]
    nc = _cache[key]
    res = run_bass_kernel_spmd(nc, in_maps, core_ids=list(range(NC_)))
    return np.concatenate(
        [res.results[c]["out"][:NLOC] for c in range(NC_)], axis=0)


def _host_prep_cached(inputs):
    return _host_prep(inputs)



# revision 37
# speedup vs baseline: 1.1077x; 1.1066x over previous
"""HGT layer kernel for 8 Trainium2 NeuronCores (Bass/Tile) — v2.

Design (vs. v1 baseline):
- dst-range sharding: core c owns dst nodes [c*6250, (c+1)*6250); edges bucketed
  per dst-owner, chunk-aligned packing (CAP=192 per 128-node chunk, NSUB=74
  subtiles per relation) so the SPMD program is static across cores.
- Compact kv table: only the ~20k unique src nodes each core actually
  references (host computes uniq + remap); k-half has NO bias (cancels in
  segment softmax), v-half keeps bv (so empty segments stay exactly 0).
- Host pre-transposes x (xgT/xqT) so phase A needs no DMA transposes.
- qa side: NO DRAM table and NO gathers — per-pair qA windows are built
  on-the-fly (PE matmul) and expanded per-edge with one-hot S^T matmuls.
- Node-major segment sums: num [node,256] + 4-wide den via the same one-hot S
  lhsT; division with free-dim broadcast; z transposed back via PE transposes
  for the folded output transform (rel_msg x Wa x alpha/R folded into wt).
- exp via Act engine broadcast to 64-wide so Y multiply runs in DVE 2x mode.
- alpha, ba folded host-side into xb_pre = (1-alpha)x + alpha*ba.
"""
import sys, types
import numpy as np
import ml_dtypes

if "antenv.axon_hooks" not in sys.modules:
    try:
        from trn_agent_boot.trn_boot import _ntff_profile_via_ctypes as _mk_hook
        _m = types.ModuleType("antenv.axon_hooks")
        _m.get_axon_ntff_profile_hook = lambda: None
        sys.modules["antenv.axon_hooks"] = _m
    except Exception:
        pass

import concourse.bass as bass
import concourse.bacc as bacc
import concourse.tile as tile
import concourse.mybir as mybir
from concourse.bass_utils import run_bass_kernel_spmd

BF16 = mybir.dt.bfloat16
F32 = mybir.dt.float32
I32 = mybir.dt.int32
BF = ml_dtypes.bfloat16
F8 = mybir.dt.float8e4
F8N = ml_dtypes.float8_e4m3
Alu = mybir.AluOpType
Act = mybir.ActivationFunctionType

N, D, R, H, DK = 50000, 256, 4, 4, 64
NC_ = 8
NLOC = N // NC_          # 6250
CH = 128
NCHUNK = 49
NPAIR = 25
NSUB = 75                # 74 used + 1 pad so every pair has 3
CAP = 192
NQ = 6400                # padded own-node count
SQRT_DK = 8.0

_cache: dict = {}


def pb4(t):
    """[128, 3, 256] -> [128, 3, 4] view picking element 0 of each 64-block."""
    return t.rearrange("p s (h d) -> p s h d", h=4)[:, :, :, 0]



# ---------------------------------------------------------------- host prep
def _pack_edges(src, dst, core, uniq):
    """Chunk-aligned packing (as v1): returns okv [128, NSUB] int32 (compact
    kv rows), S [128, NSUB, 128] bf16, ST [128, NSUB, 128] bf16."""
    sel = (dst >= core * NLOC) & (dst < (core + 1) * NLOC)
    es = src[sel].astype(np.int64)
    ed = (dst[sel] - core * NLOC).astype(np.int64)
    chunk = ed >> 7
    order = np.lexsort((es, chunk))
    es, ed, chunk = es[order], ed[order], chunk[order]
    counts = np.bincount(chunk, minlength=NCHUNK)
    if counts.max() > CAP:
        raise RuntimeError(f"chunk overflow: {counts.max()} > {CAP}")
    starts = np.zeros(NCHUNK, np.int64)
    starts[1:] = np.cumsum(counts)[:-1]
    slot = np.arange(len(ed)) - starts[chunk]
    P = chunk >> 1
    even = (chunk & 1) == 0
    sub = np.where(even,
                   np.where(slot < 128, 3 * P, 3 * P + 1),
                   np.where(slot < 64, 3 * P + 1, 3 * P + 2))
    part = np.where(even,
                    np.where(slot < 128, slot, slot - 128),
                    np.where(slot < 64, 64 + slot, slot - 64))
    okv = np.zeros((128, NSUB), np.int32)
    S = np.zeros((128, NSUB, 128), np.float32)
    kvrow = np.searchsorted(uniq, es).astype(np.int32)
    okv[part, sub] = kvrow
    S[part, sub, ed & 127] = 1.0
    ST = S.transpose(2, 1, 0).copy()    # [node, sub, edge]
    return okv, S, ST


def _host_prep(inputs):
    x = np.asarray(inputs["x"], np.float32)
    Wk = np.asarray(inputs["Wk"], np.float32)
    Wq, bq = np.asarray(inputs["Wq"], np.float32), np.asarray(inputs["bq"], np.float32)
    Wv, bv = np.asarray(inputs["Wv"], np.float32), np.asarray(inputs["bv"], np.float32)
    Wa, ba = np.asarray(inputs["Wa"], np.float32), np.asarray(inputs["ba"], np.float32)
    rel_att = np.asarray(inputs["rel_att"], np.float32)
    rel_msg = np.asarray(inputs["rel_msg"], np.float32)
    rel_pri = np.asarray(inputs["rel_pri"], np.float32)
    skip = np.asarray(inputs["skip"], np.float32)
    esrc = np.asarray(inputs["edge_src"])
    edst = np.asarray(inputs["edge_dst"])

    alpha = float(1.0 / (1.0 + np.exp(-skip[0])))

    # k weights fp8 (DoubleRow), v weights bf16, v bias row
    wk8 = Wk.T.reshape(2, 128, D).transpose(1, 0, 2).astype(F8N).copy()
    wv16 = Wv.T.reshape(2, 128, D).transpose(1, 0, 2).astype(BF).copy()
    bvv = bv[None, :].astype(BF)

    # qa fold
    WqT4 = Wq.T.reshape(D, H, DK)
    As = rel_att * (rel_pri[:, :, None, None] / SQRT_DK)
    Gq = np.einsum("ihf,rhdf->rihd", WqT4, As).reshape(R, D, D)
    bqa_full = np.einsum("hf,rhdf->rhd", bq.reshape(H, DK), As).reshape(R, D)
    wqa = np.stack([
        np.concatenate([Gq[2 * p], Gq[2 * p + 1]], axis=1).reshape(2, 128, 512)
        for p in range(2)]).transpose(2, 0, 1, 3).astype(F8N).copy()  # [128, pr, ks, 512]
    bqa = np.stack([
        np.concatenate([bqa_full[2 * p], bqa_full[2 * p + 1]])
        for p in range(2)])[None, :, :].astype(BF)                    # [1, pr, 512]

    # output transform fold (alpha included)
    Wa4 = Wa.reshape(D, H, DK)
    wt = (alpha * np.einsum("rhdf,ohf->rhdo", rel_msg, Wa4) / R).reshape(R, 2, 128, D)
    wt = wt.transpose(2, 0, 1, 3).astype(BF).copy()                   # [128, R, ks, 256]

    ident = np.eye(128, dtype=BF)

    common = dict(wk8=wk8, wv16=wv16, bvv=bvv, wqa=wqa, bqa=bqa, wt=wt,
                  ident=ident)

    # per-core uniq determines NGP (must be static across cores)
    cores = []
    for c in range(NC_):
        srcs = []
        for r in range(R):
            sel = (edst[r] >= c * NLOC) & (edst[r] < (c + 1) * NLOC)
            srcs.append(esrc[r][sel])
        uniq = np.unique(np.concatenate(srcs))
        cores.append(uniq)
    NGP = max(len(u) for u in cores)
    NGP = ((NGP + 2047) // 2048) * 2048

    in_maps = []
    cmax = np.zeros(NPAIR, np.int64)
    prepped = []
    for c in range(NC_):
        uniq = cores[c]
        okv = np.zeros((128, R, NSUB), np.int32)
        S = np.zeros((128, R, NSUB, 128), np.float32)
        ST = np.zeros((128, R, NSUB, 128), np.float32)
        for r in range(R):
            okv[:, r], S[:, r], ST[:, r] = _pack_edges(esrc[r], edst[r], c, uniq)
        # first-use pair per table row; stable-reorder rows by it
        first_use = np.full(len(uniq), NPAIR - 1, np.int64)
        for P in range(NPAIR - 1, -1, -1):
            hi = min(3 * P + 3, NSUB)
            rows = okv[:, :, 3 * P:hi].ravel()
            first_use[rows] = P
        order = np.argsort(first_use, kind="stable")
        inv = np.empty_like(order)
        inv[order] = np.arange(len(order))
        okv = inv[okv].astype(np.int32)
        uniq = uniq[order]
        cnt = np.bincount(first_use, minlength=NPAIR)
        cmax = np.maximum(cmax, np.cumsum(cnt))
        prepped.append((uniq, okv, S, ST))
    CPREF = [int(v) for v in cmax]
    for c in range(NC_):
        uniq, okv, S, ST = prepped[c]
        xgT = np.zeros((D, NGP), BF)
        xgT[:, :len(uniq)] = x[uniq].T.astype(BF)
        xgT8 = np.zeros((D, NGP), F8N)
        xgT8[:, :len(uniq)] = x[uniq].T.astype(F8N)
        xqT8 = np.zeros((D, NQ), F8N)
        xqT8[:, :NLOC] = x[c * NLOC:(c + 1) * NLOC].T.astype(F8N)
        xb = np.zeros((NQ, D), BF)
        xb[:NLOC] = ((1.0 - alpha) * x[c * NLOC:(c + 1) * NLOC]
                     + alpha * ba).astype(BF)
        # pair-major S/ST split into 4 full-128 pieces:
        # j0=sub0(ch0), j1=sub1 rows[0:64) (ch0), j2=sub1 rows[64:128) (ch1),
        # j3=sub2(ch1).  [NPAIR, 128, R, 4, 128]
        sm = np.zeros((NPAIR, 128, R, 4, 128), F8N)
        st = np.zeros((NPAIR, 128, R, 4, 128), F8N)
        for P in range(NPAIR):
            s0, s1, s2 = 3 * P, 3 * P + 1, 3 * P + 2
            sm[P, :, :, 0] = S[:, :, s0].astype(F8N)
            sm[P, 0:64, :, 1] = S[0:64, :, s1].astype(F8N)
            sm[P, 64:128, :, 2] = S[64:128, :, s1].astype(F8N)
            if s2 < NSUB:
                sm[P, :, :, 3] = S[:, :, s2].astype(F8N)
            st[P, :, :, 0] = ST[:, :, s0].astype(F8N)
            st[P, :, :, 1] = (ST[:, :, s1] * (np.arange(128) < 64)).astype(F8N)
            st[P, :, :, 2] = (ST[:, :, s1] * (np.arange(128) >= 64)).astype(F8N)
            if s2 < NSUB:
                st[P, :, :, 3] = ST[:, :, s2].astype(F8N)
        # dma_gather int16 indices: per pair, linear order i = (r*3+s)*128 + p,
        # wrapped [16, 96] (idx i at [i%16, i//16]) then replicated to 128 rows
        okv16 = np.zeros((NPAIR, 128, 96), np.int16)
        for P in range(NPAIR):
            linear = okv[:, :, 3 * P:3 * P + 3].transpose(1, 2, 0).reshape(1536)
            blk = linear.reshape(96, 16).T.astype(np.int16)     # [16, 96]
            okv16[P] = np.tile(blk, (8, 1))
        in_maps.append(dict(common, xgT=xgT, xgT8=xgT8, xqT8=xqT8, xb=xb,
                            okv=okv, okv16=okv16, smat=sm, stmat=st))
    return in_maps, (NGP, tuple(CPREF))


# ---------------------------------------------------------------- device build
def _build_nc(sig):
    NGP, CPREF = sig
    nc = bacc.Bacc("TRN2", target_bir_lowering=False, debug=False, num_devices=NC_)
    dt = nc.dram_tensor
    xgT_in = dt("xgT", [D, NGP], BF16, kind="ExternalInput").ap()
    xgT8_in = dt("xgT8", [D, NGP], F8, kind="ExternalInput").ap()
    xqT8_in = dt("xqT8", [D, NQ], F8, kind="ExternalInput").ap()
    xb_in = dt("xb", [NQ, D], BF16, kind="ExternalInput").ap()
    wk8_in = dt("wk8", [128, 2, D], F8, kind="ExternalInput").ap()
    wv16_in = dt("wv16", [128, 2, D], BF16, kind="ExternalInput").ap()
    bvv_in = dt("bvv", [1, D], BF16, kind="ExternalInput").ap()
    wqa_in = dt("wqa", [128, 2, 2, 512], F8, kind="ExternalInput").ap()
    bqa_in = dt("bqa", [1, 2, 512], BF16, kind="ExternalInput").ap()
    wt_in = dt("wt", [128, R, 2, D], BF16, kind="ExternalInput").ap()
    ident_in = dt("ident", [128, 128], BF16, kind="ExternalInput").ap()
    okv_in = dt("okv", [128, R, NSUB], I32, kind="ExternalInput").ap()
    okv16_in = dt("okv16", [NPAIR, 16, 96], mybir.dt.int16,
                  kind="ExternalInput").ap()
    smat_in = dt("smat", [NPAIR, 128, R, 4, 128], F8, kind="ExternalInput").ap()
    stmat_in = dt("stmat", [NPAIR, 128, R, 4, 128], F8, kind="ExternalInput").ap()
    out = dt("out", [NQ, D], F32, kind="ExternalOutput").ap()

    kvt = dt("kvt", [NGP, 768], mybir.dt.uint8, kind="Internal").ap()
    NGROUP = (CPREF[-1] + 511) // 512

    with tile.TileContext(nc) as tc:
        with tc.tile_pool(name="const", bufs=1) as cp:
            wk8_t = cp.tile([128, 2, D], F8)
            nc.sync.dma_start(wk8_t[:], wk8_in[:])
            wv16_t = cp.tile([128, 2, D], BF16)
            nc.sync.dma_start(wv16_t[:], wv16_in[:])
            bvv_t = cp.tile([1, D], BF16)
            nc.sync.dma_start(bvv_t[:], bvv_in[:])
            wqa_t = cp.tile([128, 2, 2, 512], F8)
            nc.sync.dma_start(wqa_t[:], wqa_in[:])
            bqa_t = cp.tile([1, 2, 512], BF16)
            nc.sync.dma_start(bqa_t[:], bqa_in[:])
            wt_t = cp.tile([128, R, 2, D], BF16)
            nc.sync.dma_start(wt_t[:], wt_in[:])
            ident_t = cp.tile([128, 128], BF16)
            nc.sync.dma_start(ident_t[:], ident_in[:])
            okv_t = cp.tile([128, R, NSUB], I32)
            nc.sync.dma_start(okv_t[:], okv_in[:])
            okv16_t = cp.tile([16, NPAIR, 96], mybir.dt.int16)
            nc.sync.dma_start(okv16_t[:],
                              okv16_in.rearrange("n p s -> p n s"))
            ones_bf = cp.tile([1, 128], BF16)
            nc.vector.memset(ones_bf[:], 1.0)
            xqT_t = cp.tile([128, 2, NQ], F8)
            nc.sync.dma_start(
                xqT_t[:], xqT8_in.rearrange("(ks p) n -> p ks n", ks=2))

            with (
                tc.tile_pool(name="xload", bufs=2) as xp,
                tc.tile_pool(name="kvsb", bufs=3) as kvp,
                tc.tile_pool(name="sst", bufs=3) as sp,
                tc.tile_pool(name="gath", bufs=3) as gp,
                tc.tile_pool(name="qaws", bufs=2) as qwp,
                tc.tile_pool(name="edve", bufs=4) as ep,
                tc.tile_pool(name="zts", bufs=6) as zp,
                tc.tile_pool(name="fin", bufs=3) as fp,
                tc.tile_pool(name="psQA", bufs=2, space="PSUM") as psQA,
                tc.tile_pool(name="psQB2", bufs=2, space="PSUM") as psQB2,
                tc.tile_pool(name="psNum", bufs=2, space="PSUM") as psNum,
                tc.tile_pool(name="psR", bufs=2, space="PSUM") as psR,
            ):
                gsem = nc.alloc_semaphore("swdge_dma")
                bstate = {"t": 0, "xT": None, "kvs": None}
                NTILE = NGROUP * 4

                def emit_tiles(ntiles):
                    done = 0
                    while done < ntiles and bstate["t"] < NTILE:
                        ti = bstate["t"]
                        g, t = divmod(ti, 4)
                        w, gi = divmod(g, 4)
                        if gi == 0 and t == 0:
                            xT = xp.tile([128, 2, 2048], BF16, tag="xT", name=f"xT{w}")
                            nc.sync.dma_start(
                                xT[:], xgT_in[:, w * 2048:(w + 1) * 2048]
                                .rearrange("(ks p) n -> p ks n", ks=2))
                            xT8 = xp.tile([128, 2, 2048], F8, tag="xT8",
                                          name=f"xT8{w}")
                            nc.sync.dma_start(
                                xT8[:], xgT8_in[:, w * 2048:(w + 1) * 2048]
                                .rearrange("(ks p) n -> p ks n", ks=2))
                            bstate["xT"] = (xT, xT8)
                        if t == 0:
                            bstate["kvs"] = kvp.tile(
                                [128, 4, 768], mybir.dt.uint8, tag="kvs",
                                name=f"kvs{g}")
                        (xT, xT8), kvs = bstate["xT"], bstate["kvs"]
                        nt = gi * 4 + t
                        if t % 2 == 0:
                            bstate["pk"] = psR.tile([128, 512], F32, tag="mt",
                                                    name=f"pk{g}_{t}")
                            bstate["pv"] = psR.tile([128, 512], F32, tag="mt",
                                                    name=f"pv{g}_{t}")
                        pk, pv = bstate["pk"], bstate["pv"]
                        half = (t % 2) * 256
                        nc.tensor.matmul(
                            pk[:, half:half + 256], xT8[:, :, nt * 128:(nt + 1) * 128],
                            wk8_t[:], start=True, stop=True,
                            perf_mode=mybir.MatmulPerfMode.DoubleRow)
                        for ks in range(2):
                            nc.tensor.matmul(
                                pv[:, half:half + 256],
                                xT[:, ks, nt * 128:(nt + 1) * 128],
                                wv16_t[:, ks], start=(ks == 0), stop=False)
                        nc.tensor.matmul(pv[:, half:half + 256], ones_bf[:],
                                         bvv_t[:], start=False, stop=True)
                        if t % 2 == 1:
                            nc.scalar.copy(
                                kvs[:, t - 1:t + 1, 0:256].bitcast(F8),
                                pk[:].rearrange("p (s f) -> p s f", s=2))
                            nc.scalar.copy(
                                kvs[:, t - 1:t + 1, 256:768].bitcast(BF16),
                                pv[:].rearrange("p (s f) -> p s f", s=2))
                        if t == 3:
                            base = g * 512
                            nc.sync.dma_start(
                                kvt[base:base + 512].rearrange("(s p) f -> p s f", s=4),
                                kvs[:])
                        bstate["t"] += 1
                        done += 1

                def emit_groups(upto):
                    need = min(upto, NGROUP) * 4
                    if bstate["t"] < need:
                        emit_tiles(need - bstate["t"])

                GATHER_MODE = "split"

                def fetch_kvg(P):
                    t = gp.tile([128, R, 3, 768], mybir.dt.uint8, tag="kvg",
                                name=f"kvg{P}")
                    if GATHER_MODE == "swdge":
                        nc.gpsimd.dma_gather(
                            out_ap=t.rearrange("p r s f -> p (r s) f"),
                            in_ap=kvt[0:CPREF[P]],
                            idxs_ap=okv16_t[:, P, :],
                            num_idxs=1536, num_idxs_reg=1536, elem_size=768,
                            single_packet=False)
                    elif GATHER_MODE == "batched":
                        nc.gpsimd.indirect_dma_start(
                            out=t.rearrange("p r s f -> p (r s) f"), out_offset=None,
                            in_=kvt[0:CPREF[P]],
                            in_offset=bass.IndirectOffsetOnAxis(
                                ap=okv_t[:, :, 3 * P:3 * P + 3], axis=0))
                    elif GATHER_MODE == "perrel":
                        ns_ = 2 if P == NPAIR - 1 else 3
                        for r_ in range(R):
                            nc.gpsimd.indirect_dma_start(
                                out=t[:, r_, 0:ns_, :], out_offset=None,
                                in_=kvt[0:CPREF[P]],
                                in_offset=bass.IndirectOffsetOnAxis(
                                    ap=okv_t[:, r_, 3 * P:3 * P + ns_], axis=0))
                    else:
                        ns_ = 2 if P == NPAIR - 1 else 3
                        for r_ in range(R):
                            for s_ in range(ns_):
                                nc.gpsimd.indirect_dma_start(
                                    out=t[:, r_, s_, :], out_offset=None,
                                    in_=kvt[0:CPREF[P]],
                                    in_offset=bass.IndirectOffsetOnAxis(
                                        ap=okv_t[:, r_, 3 * P + s_:3 * P + s_ + 1],
                                        axis=0))
                    return t

                def build_pairres(P):
                    nch_ = 1 if P == NPAIR - 1 else 2
                    qaw = qwp.tile([128, 2, 1024], BF16, tag="qaw")
                    for ch in range(nch_):
                        nb = P * 256 + ch * 128
                        for pr in range(2):
                            qab = psR.tile([128, 512], F32, tag="mt",
                                           name=f"qab{P}_{ch}_{pr}")
                            nc.tensor.matmul(
                                qab[:], xqT_t[:, :, nb:nb + 128],
                                wqa_t[:, pr], start=True, stop=False,
                                perf_mode=mybir.MatmulPerfMode.DoubleRow)
                            nc.tensor.matmul(qab[:], ones_bf[:], bqa_t[:, pr],
                                             start=False, stop=True)
                            nc.scalar.copy(qaw[:, ch, pr * 512:(pr + 1) * 512],
                                           qab[:])
                    S_t = sp.tile([128, R, 4, 128], F8, tag="S")
                    nc.sync.dma_start(S_t[:], smat_in[P])
                    ST_t = sp.tile([128, R, 4, 128], F8, tag="ST")
                    nc.sync.dma_start(ST_t[:], stmat_in[P])
                    xbt = fp.tile([128, 2, 256], BF16, tag="xbt")
                    nc.sync.dma_start(
                        xbt[:, :nch_],
                        xb_in[P * 256:P * 256 + nch_ * 128]
                        .rearrange("(c p) f -> p c f", c=nch_))
                    return qaw, S_t, ST_t, xbt

                bstate["kvq"] = []
                bstate["pres"] = None
                for P in range(NPAIR):
                    last = (P == NPAIR - 1)
                    ns = 2 if last else 3
                    nch = 1 if last else 2
                    emit_groups((CPREF[min(P + 3, NPAIR - 1)] + 511) // 512)
                    if last:
                        emit_groups(NGROUP)
                    while len(bstate["kvq"]) < min(3, NPAIR - P):
                        bstate["kvq"].append(fetch_kvg(P + len(bstate["kvq"])))
                    kvg = bstate["kvq"].pop(0)
                    if bstate["pres"] is None:
                        bstate["pres"] = build_pairres(P)
                    qaw, S_t, ST_t, xbt = bstate["pres"]

                    zts = []

                    def stageA(r):
                        qlo = (r // 2) * 512 + (r % 2) * 256

                        def qwsl(ch, lo=qlo):
                            return qaw[:, ch, lo:lo + 256]
                        kv_g = kvg[:, r]
                        qa01 = psQA.tile([128, 512], F32, tag="qa01", name=f"qa01_{P}_{r}")
                        qa2d = psQB2.tile([128, 512], F32, tag="qa2d", name=f"qa2d_{P}_{r}")
                        nc.tensor.matmul(qa01[:, 0:256], ST_t[:, r, 0, :], qwsl(0),
                                         start=True, stop=True)
                        nc.tensor.matmul(qa01[:, 256:512], ST_t[:, r, 1, :],
                                         qwsl(0), start=True, stop=False)
                        nc.tensor.matmul(qa01[:, 256:512], ST_t[:, r, 2, :],
                                         qwsl(1 if not last else 0),
                                         start=False, stop=True)
                        if not last:
                            nc.tensor.matmul(qa2d[:, 0:256], ST_t[:, r, 3, :], qwsl(1),
                                             start=True, stop=True)
                        return kv_g, qa01, qa2d

                    def stageB(r, kv_g, qa01, qa2d):
                        prodb = ep.tile([128, 3, 256], BF16, tag="prodb")
                        nc.vector.tensor_tensor(
                            out=prodb[:, :2], in0=kv_g[:, :2, 0:256].bitcast(F8),
                            in1=qa01[:].rearrange("p (s f) -> p s f", f=256),
                            op=Alu.mult)
                        if not last:
                            nc.vector.tensor_tensor(
                                out=prodb[:, 2], in0=kv_g[:, 2, 0:256].bitcast(F8),
                                in1=qa2d[:, 0:256], op=Alu.mult)
                        pr4 = prodb.rearrange("p s (h d) -> p s h d", h=4)
                        fold = ep.tile([128, 3, 4, 32], BF16, tag="fold")
                        nc.vector.tensor_tensor(
                            out=fold[:, :ns], in0=pr4[:, :ns, :, 0:32],
                            in1=pr4[:, :ns, :, 32:64], op=Alu.add)
                        attf = ep.tile([128, 3, 4], F32, tag="attf")
                        nc.vector.tensor_reduce(
                            attf[:, :ns], fold[:, :ns],
                            axis=mybir.AxisListType.X, op=Alu.add)
                        pb16 = ep.tile([128, 3, 4, 16], BF16, tag="pb16")
                        nc.scalar.activation(
                            pb16[:, :ns],
                            attf[:, :ns, :, None].to_broadcast([128, ns, 4, 16]),
                            Act.Exp)
                        # den (node-major) then per-edge 1/den expansion
                        nc.tensor.matmul(qa2d[:, 256:260], S_t[:, r, 0, :],
                                         pb16[:, 0, :, 0], start=True, stop=False)
                        nc.tensor.matmul(qa2d[:, 256:260], S_t[:, r, 1, :],
                                         pb16[:, 1, :, 0], start=False, stop=True)
                        if not last:
                            nc.tensor.matmul(qa2d[:, 260:264], S_t[:, r, 2, :],
                                             pb16[:, 1, :, 0], start=True, stop=False)
                            nc.tensor.matmul(qa2d[:, 260:264], S_t[:, r, 3, :],
                                             pb16[:, 2, :, 0], start=False, stop=True)
                        rdenf = ep.tile([128, 8], F32, tag="rdenf")
                        nc.vector.tensor_scalar_max(rdenf[:, :nch * 4],
                                                    qa2d[:, 256:256 + nch * 4], 1e-9)
                        rden = ep.tile([128, 8], BF16, tag="rden")
                        with nc.allow_low_precision("1/den in bf16 is within tol"):
                            nc.vector.reciprocal(rden[:, :nch * 4],
                                                 rdenf[:, :nch * 4])
                        rdE = qa2d[:, 272:284].rearrange("p (s h) -> p s h", s=3)
                        nc.tensor.matmul(rdE[:, 0], ST_t[:, r, 0, :], rden[:, 0:4],
                                         start=True, stop=True)
                        nc.tensor.matmul(rdE[:, 1], ST_t[:, r, 1, :], rden[:, 0:4],
                                         start=True, stop=last)
                        if not last:
                            nc.tensor.matmul(rdE[:, 1], ST_t[:, r, 2, :],
                                             rden[:, 4:8], start=False, stop=True)
                            nc.tensor.matmul(rdE[:, 2], ST_t[:, r, 3, :],
                                             rden[:, 4:8], start=True, stop=True)
                        W = ep.tile([128, 3, 4, 16], BF16, tag="W")
                        nc.vector.tensor_tensor(
                            out=W[:, :ns], in0=pb16[:, :ns],
                            in1=rdE[:, :ns, :, None].to_broadcast([128, ns, 4, 16]),
                            op=Alu.mult)
                        Y = ep.tile([128, 3, 256], BF16, tag="Y")
                        nc.vector.tensor_tensor(
                            out=Y[:, :ns].rearrange(
                                "p s (h o e) -> p s h o e", h=4, e=16),
                            in0=kv_g[:, :ns, 256:768].bitcast(BF16).rearrange(
                                "p s (h o e) -> p s h o e", h=4, e=16),
                            in1=W[:, :ns, :, None, :]
                                .to_broadcast([128, ns, 4, 4, 16]),
                            op=Alu.mult)

                        # hd-major weighted segment sum: numT[hd, u]
                        numT = psNum.tile([128, 2, 2, 128], F32, tag="num")
                        for half in range(2):
                            ysl = [Y[:, s_, half * 128:(half + 1) * 128]
                                   for s_ in range(3)]
                            nc.tensor.matmul(numT[:, half, 0], ysl[0],
                                             S_t[:, r, 0, :], start=True, stop=False)
                            nc.tensor.matmul(numT[:, half, 0], ysl[1],
                                             S_t[:, r, 1, :], start=False, stop=True)
                            if not last:
                                nc.tensor.matmul(numT[:, half, 1], ysl[1],
                                                 S_t[:, r, 2, :], start=True,
                                                 stop=False)
                                nc.tensor.matmul(numT[:, half, 1], ysl[2],
                                                 S_t[:, r, 3, :], start=False,
                                                 stop=True)
                        zS = zp.tile([128, 2, 2, 128], BF16, tag="ztS")
                        nc.scalar.copy(zS[:, :, :nch], numT[:, :, :nch])
                        zts.append(zS)
                        emit_tiles(2 if P < 10 else 1)

                    for r in range(R):
                        stageB(r, *stageA(r))
                        if r == 1 and not last:
                            bstate["pres"] = build_pairres(P + 1)

                    # transform + blend (xb added via identity matmul)
                    pt = psR.tile([128, 512], F32, tag="mt", name=f"pt{P}")
                    for ch in range(nch):
                        for i in range(8):
                            r, ks = i // 2, i % 2
                            nc.tensor.matmul(
                                pt[:, ch * 256:(ch + 1) * 256],
                                zts[r][:, ks, ch],
                                wt_t[:, r, ks], start=(i == 0), stop=False)
                        nc.tensor.matmul(
                            pt[:, ch * 256:(ch + 1) * 256], ident_t[:],
                            xbt[:, ch], start=False, stop=True)
                    o = fp.tile([128, 2, 256], F32, tag="o")
                    nc.scalar.copy(o[:, :nch], pt[:, :nch * 256]
                                   .rearrange("p (c f) -> p c f", f=256))
                    nc.sync.dma_start(
                        out[P * 256:P * 256 + nch * 128]
                        .rearrange("(c p) f -> p c f", c=nch),
                        o[:, :nch])
    nc.compile()
    return nc


def kernel(**inputs):
    in_maps, sig = _host_prep_cached(inputs)
    key = ("nc", sig)
    if key not in _cache:
        _cache[key] = _build_nc(sig)
        _cache["nc"] = _cache[key]
    nc = _cache[key]
    res = run_bass_kernel_spmd(nc, in_maps, core_ids=list(range(NC_)))
    return np.concatenate(
        [res.results[c]["out"][:NLOC] for c in range(NC_)], axis=0)


def _host_prep_cached(inputs):
    return _host_prep(inputs)

